# revision 12
# baseline (speedup 1.0000x reference)
"""Tricubic-spline PSF sampling kernel for Trainium2 (8 NeuronCores).

Problem: nn_CubicSplinePSF — for each of 512 emitters, evaluate a tricubic
spline on an [8, 20, 20] voxel grid, normalize per (emitter, z-plane),
scale by intensity and add background.

Key structural insight: with pos in [0, 1), the per-voxel floor cell indices
form a FIXED pattern (iz = z+27, iy = y+9, ix = x+9) and the fractional
offsets are per-emitter constants. So the irregular 64-wide gather collapses
to a fixed slice of the coefficient table, and the evaluation becomes

    out[i, z, y, x] = sum_k basis_z[i, k] * C_z[(y,x), k]

with C_z = coefs[27+z, 9:29, 9:29, :] reshaped to [400, 64] and basis_z the
64-term tricubic monomial basis (outer product of [1,d,d^2,d^3] per axis).
One z-plane per NeuronCore (8 planes / 8 cores) — normalization is per
(emitter, plane) so there is no cross-core communication. The per-emitter
sum needed for normalization comes for free as a 401st matmul column equal
to the row-sum of C_z.

Emitters whose floor pattern deviates (pos component exactly 0 / within an
ULP of it — probability ~1e-4) are computed exactly on the host and patched
into the result.
"""

import sys

if "/opt/trn_rl_repo" not in sys.path:
    sys.path.insert(0, "/opt/trn_rl_repo")

import numpy as np

import concourse.bacc as bacc
import concourse.bass as bass
import concourse.tile as tile
from concourse import mybir
from concourse.bass_utils import run_bass_kernel_spmd

N = 512
ZPLANES = 8
ROISIZE = 20
NVOX = ROISIZE * ROISIZE  # 400 voxels per plane
NCHUNK = N // 128  # 4 partition chunks of emitters
F32 = mybir.dt.float32
F32R = mybir.dt.float32r

TRACE = False  # set kernel.TRACE = True (from test.py) to capture an NTFF profile
LAST_RESULTS = None  # BassKernelResults of the most recent run (for profiling)

_NC = None  # cached Bass module


def _build_bass():
    nc = bacc.Bacc("TRN2", target_bir_lowering=False, debug=False)
    # Packed [bT | cT]: basis-transpose [64, 512] and coef-slice-transpose
    # (with row-sum column) [64, 401] in one tensor → one DMA → one wait
    # on the first matmul (LDW has a tiny sync-wait budget).
    w = nc.dram_tensor("w", [64, N + NVOX + 2], F32R, kind="ExternalInput").ap()
    inten = nc.dram_tensor("inten", [NCHUNK, 128, 1], F32, kind="ExternalInput").ap()
    bg = nc.dram_tensor("bg", [NCHUNK, 128, 1], F32, kind="ExternalInput").ap()
    out = nc.dram_tensor("out", [N, NVOX], F32, kind="ExternalOutput").ap()

    with tile.TileContext(nc) as tc:
        with (
            tc.tile_pool(name="const", bufs=1) as cpool,
            tc.tile_pool(name="io", bufs=4) as iopool,
            tc.tile_pool(name="small", bufs=8) as spool,
            tc.tile_pool(name="ps", bufs=4, space="PSUM") as pspool,
        ):
            w_sb = cpool.tile([64, N + NVOX + 2], F32R)
            nc.sync.dma_start(w_sb[:], w)

            for m in range(NCHUNK):
                it = spool.tile([128, 1], F32, tag="it")
                nc.sync.dma_start(it[:], inten[m])
                bgt = spool.tile([128, 1], F32, tag="bgt")
                nc.sync.dma_start(bgt[:], bg[m])

                ps = pspool.tile([128, NVOX + 2], F32)
                nc.tensor.matmul(
                    ps[:],
                    lhsT=w_sb[:, m * 128 : (m + 1) * 128],
                    rhs=w_sb[:, N : N + NVOX + 2],  # fp32r needs even N: 402 = 400 vox + sum + pad
                    start=True,
                    stop=True,
                )

                inv = spool.tile([128, 1], F32, tag="inv")
                nc.vector.reciprocal(inv[:], ps[:, NVOX : NVOX + 1])
                sc = spool.tile([128, 1], F32, tag="sc")
                nc.vector.tensor_mul(sc[:], inv[:], it[:])

                ob = iopool.tile([128, NVOX], F32)
                nc.scalar.activation(
                    ob[:],
                    ps[:, 0:NVOX],
                    mybir.ActivationFunctionType.Identity,
                    bias=bgt[:],
                    scale=sc[:],
                )
                nc.sync.dma_start(out[m * 128 : (m + 1) * 128, :], ob[:])
    nc.compile()
    return nc


def _frac_grids(pos):
    """Replicate the reference's f32 coordinate arithmetic exactly.

    Returns floor-index and fractional-part grids per axis:
    (iz, dz) of shape [N, 8] and (iy, dy), (ix, dx) of shape [N, 20].
    """
    f32 = np.float32
    z = np.arange(ZPLANES, dtype=f32)
    r = np.arange(ROISIZE, dtype=f32)
    pz = z[None, :] - pos[:, 2:3] + f32(28.0)
    py = r[None, :] - pos[:, 0:1] + f32(10.0)
    px = r[None, :] - pos[:, 1:2] + f32(10.0)
    fz, fy, fx = np.floor(pz), np.floor(py), np.floor(px)
    return (fz, pz - fz), (fy, py - fy), (fx, px - fx)


def _exact_rows(rows, pos, intensities, backgrounds, coefs):
    """Bit-faithful numpy replication of the reference for a few emitters."""
    f32 = np.float32
    (fz, dz), (fy, dy), (fx, dx) = _frac_grids(pos[rows])
    iz = np.clip(fz.astype(np.int64), 0, 63)
    iy = np.clip(fy.astype(np.int64), 0, 39)
    ix = np.clip(fx.astype(np.int64), 0, 39)
    e = np.arange(4)
    n = len(rows)
    out = np.empty((n, ZPLANES, ROISIZE, ROISIZE), f32)
    for j in range(n):
        c = coefs[
            iz[j][:, None, None], iy[j][None, :, None], ix[j][None, None, :]
        ]  # [8,20,20,64]
        bz = (dz[j][:, None] ** e).astype(f32)  # [8,4]
        by = (dy[j][:, None] ** e).astype(f32)  # [20,4]
        bx = (dx[j][:, None] ** e).astype(f32)  # [20,4]
        basis = (
            bz[:, None, None, :, None, None]
            * by[None, :, None, None, :, None]
            * bx[None, None, :, None, None, :]
        ).reshape(ZPLANES, ROISIZE, ROISIZE, 64)
        vals = (c * basis).sum(axis=-1, dtype=f32)
        s = vals.sum(axis=(1, 2), keepdims=True, dtype=f32)
        out[j] = vals / s * intensities[rows[j]][:, None, None] + backgrounds[rows[j]][
            :, None, None
        ]
    return out


def kernel(pos, intensities, backgrounds, coefs):
    global _NC, LAST_RESULTS
    f32 = np.float32
    pos = np.asarray(pos, f32)
    intensities = np.asarray(intensities, f32)
    backgrounds = np.asarray(backgrounds, f32)
    coefs = np.asarray(coefs, f32)

    (fz, dz), (fy, dy), (fx, dx) = _frac_grids(pos)
    zi = np.arange(ZPLANES, dtype=f32)
    ri = np.arange(ROISIZE, dtype=f32)
    bad = (
        (fz != zi[None, :] + 27).any(axis=1)
        | (fy != ri[None, :] + 9).any(axis=1)
        | (fx != ri[None, :] + 9).any(axis=1)
    )

    # Host prep: fixed coefficient slice (transposed, with row-sum column)
    # and the per-(core, emitter) 64-term monomial basis, transposed.
    C = coefs[27:35, 9:29, 9:29, :].reshape(ZPLANES, NVOX, 64)
    e = np.arange(4)
    by = (dy[:, 0:1] ** e).astype(f32)  # [N,4]
    bx = (dx[:, 0:1] ** e).astype(f32)  # [N,4]
    byx = (by[:, :, None] * bx[:, None, :]).reshape(N, 16)  # [N,16]

    in_maps = []
    for z in range(ZPLANES):
        bz = (dz[:, z : z + 1] ** e).astype(f32)  # [N,4]
        basis = (bz[:, :, None] * byx[:, None, :]).reshape(N, 64)
        ct = C[z].T  # [64, 400]
        w = np.empty((64, N + NVOX + 2), f32)
        w[:, :N] = basis.T
        w[:, N : N + NVOX] = ct
        w[:, N + NVOX] = ct.astype(np.float64).sum(axis=1)
        w[:, N + NVOX + 1] = 0.0
        in_maps.append(
            {
                "w": w,
                "inten": np.ascontiguousarray(
                    intensities[:, z].reshape(NCHUNK, 128, 1)
                ),
                "bg": np.ascontiguousarray(
                    backgrounds[:, z].reshape(NCHUNK, 128, 1)
                ),
            }
        )

    if _NC is None:
        _NC = _build_bass()
    res = run_bass_kernel_spmd(
        _NC, in_maps, core_ids=list(range(ZPLANES)), trace=TRACE
    )
    LAST_RESULTS = res
    out = np.stack([res.results[z]["out"] for z in range(ZPLANES)], axis=1)
    out = out.reshape(N, ZPLANES, ROISIZE, ROISIZE)

    if bad.any():
        rows = np.nonzero(bad)[0]
        out[rows] = _exact_rows(rows, pos, intensities, backgrounds, coefs)
    return out
